# revision 1
# baseline (speedup 1.0000x reference)
"""Trainium2 Bass kernel for nn_LowBitMixIn.

Computes out[b,o,t] = sum_i mixer[o,i] * x[b, perm[i], t] for
x:[16,1024,4096] f32, mixer:[1024,1024] f32 (banded: 7 cyclic
sub-diagonals), perm:[1024] int32.

Strategy: data-parallel over batch (2 batches per core, 8 cores).
The band (signed diagonal span S) lets each output tile of
M = 129-S rows be computed from a single 128-row input window with
ONE K=128 matmul per 512-column chunk. The whole device pipeline
runs bf16: the mixer weights are low-bit (powers of two, exact in
bf16), so the only loss is input/output quantization (~2.4e-3 rel
vs the 2e-2 gate) while HBM traffic halves and the PE runs 4x
faster than fp32. This flips the fp32 kernel (tensor-bound at the
roofline ridge) into a DMA-bound one and roughly halves exec time.

v2 (default): the host pre-permutes x into gathered-window order
(folding `perm` and the window overlap), so device reads are plain
contiguous SWDGE DMAs, two 1 MiB windows per instruction. Measured
pitfalls baked in: HWDGE reads mixed with SWDGE writes stall the
pipeline 2x (ring-arbitration phasing); splitting the PSUM->SBUF
casts across DVE+ACT convoys 2x; matmul N is capped at 512 by the
PSUM bank; nsp=4 write-split is pathological while nsp=5/6 are fine;
4-window (4 MiB) grouped reads cause head-of-line blocking. Two
matmuls fill one 2-bank PSUM tile so each cast moves 1024 columns,
amortizing the PSUM-read bubble; all casts run on the Scalar (ACT)
engine, which sits closer to PSUM than DVE (997 vs 1192 ns per
1024-wide cast, measured ~5% whole-kernel win).

If the mixer turns out not to be banded (verified by exact
reconstruction), a generic fp32 block-dense path is used instead.
"""

import contextlib
import os
import sys

import ml_dtypes
import numpy as np

sys.path.insert(0, "/opt/trn_rl_repo")

from concourse import bacc, bass, mybir, tile  # noqa: E402
from concourse.bass_utils import run_bass_kernel_spmd  # noqa: E402

F = 1024
T = 4096
B = 16
N_CORES = 8
B_SHARD = B // N_CORES
USE_V2 = True  # pre-permuted contiguous-read variant vs indirect gather
NCHUNK = 512  # max matmul N: PSUM bank is 512 fp32 (s3d3_mm_num_elements)

_PROGRAM_CACHE = {}
LAST_RESULTS = None  # test harness introspection (exec_time_ns etc.)
LAST_NC = None
LAST_IN_MAPS = None


def _build_banded_program(b_shard, f, t, tile_m, n_tiles, reps=1,
                          scatter_out=True, nsp=6, xt_bufs=3, ot_bufs=5,
                          ps_bufs=8):
    """scatter_out: route 3/4 of the output writes through dma_scatter_add
    on SWDGE queues 1-3 (writes into the pre-zeroed output, so add == write).
    SWDGE writes sustain ~2.7x HWDGE write bandwidth here, and per-queue
    spreading pipelines the HBM write receipts further."""
    scatter_out = scatter_out and t % (4 * SCH) == 0
    nc = bacc.Bacc(num_swdge_queues=4 if scatter_out else 1)
    x_in = nc.declare_dram_parameter("x", [b_shard * f, t], mybir.dt.bfloat16, isOutput=False)
    wts_in = nc.declare_dram_parameter(
        "wts", [n_tiles, 128, tile_m], mybir.dt.bfloat16, isOutput=False
    )
    gidx_in = nc.declare_dram_parameter(
        "gidx", [128, b_shard * n_tiles], mybir.dt.int32, isOutput=False
    )
    if scatter_out:
        sidx_in = nc.declare_dram_parameter(
            "sidx", [128, b_shard * n_tiles * 3 * 8], mybir.dt.int16,
            isOutput=False,
        )
    out_ext = nc.declare_dram_parameter(
        "out", [b_shard * f, t], mybir.dt.bfloat16, isOutput=True
    )
    n_work = b_shard * n_tiles
    with tile.TileContext(nc) as tc:
        with (
            tc.tile_pool(name="const", bufs=1) as cpool,
            tc.tile_pool(name="xp", bufs=xt_bufs) as xpool,
            tc.tile_pool(name="op", bufs=ot_bufs) as opool,
            tc.tile_pool(name="ps", bufs=ps_bufs, space="PSUM") as pspool,
        ):
            idx_tile = cpool.tile([128, b_shard * n_tiles], mybir.dt.int32)
            nc.sync.dma_start(out=idx_tile[:], in_=gidx_in[:])
            sidx_tile = None
            sc_sem = None
            if scatter_out:
                sidx_tile = cpool.tile(
                    [128, b_shard * n_tiles * 3 * 8], mybir.dt.int16
                )
                nc.sync.dma_start(out=sidx_tile[:], in_=sidx_in[:])
                sc_sem = []  # scatters are Tile-managed (DMASW lanes)
            w_tiles = []
            for p in range(n_tiles):
                wt = cpool.tile([128, tile_m], mybir.dt.bfloat16, tag=f"w{p}")
                nc.sync.dma_start(out=wt[:], in_=wts_in[p])
                w_tiles.append(wt)
            k = 0
            records = []  # per-tile scatter ordinal (None = plain DMA path)
            for _rep in range(reps):
                for bi in range(b_shard):
                    for p in range(n_tiles):
                        _emit_banded_tile(
                            nc, f, t, tile_m, bi, p, n_tiles, k,
                            idx_tile, w_tiles, xpool, opool, pspool,
                            x_in, out_ext, sidx_tile, sc_sem, records,
                            nsp,
                        )
                        k += 1
    return nc


SCH = 1024  # scatter column-chunk width


def b_shard_f_minus1(nc, f, t, x_in):
    return x_in.shape[0] - 1


def _emit_banded_tile(
    nc, f, t, tile_m, bi, p, n_tiles, k,
    idx_tile, w_tiles, xpool, opool, pspool, x_in, out_ext,
    sidx_tile, sc_sem, records, nsp=6,
):
    n_chunks = t // NCHUNK
    o0 = p * tile_m
    m_p = min(tile_m, f - o0)
    col = bi * n_tiles + p
    # partial tiles engage <16 SDMA engines per scatter, so the completion
    # sem under-increments; route them through the Tile-managed DMA path
    use_scatter = sc_sem is not None and m_p == tile_m
    k_need = m_p + (129 - tile_m) - 1  # window rows with nonzero weights
    xt = xpool.tile([128, t], mybir.dt.bfloat16, tag="xt")
    nc.gpsimd.indirect_dma_start(
        out=xt[:],
        out_offset=None,
        in_=x_in[:],
        in_offset=bass.IndirectOffsetOnAxis(
            ap=idx_tile[:, col : col + 1], axis=0
        ),
        bounds_check=b_shard_f_minus1(nc, f, t, x_in),
        oob_is_err=False,
    )
    ot = opool.tile([128, t], mybir.dt.bfloat16, tag="ot")
    for ni in range(n_chunks):
        ps = pspool.tile([128, NCHUNK], mybir.dt.float32)
        nc.tensor.matmul(
            out=ps[:m_p, :],
            lhsT=w_tiles[p][:k_need, :m_p],
            rhs=xt[:k_need, ni * NCHUNK : (ni + 1) * NCHUNK],
            start=True,
            stop=True,
        )
        # single cast engine: mixing DVE+ACT here measured 2.2x slower
        # (semaphore convoys between the in-order engine queues)
        nc.vector.tensor_copy(
            out=ot[:m_p, ni * NCHUNK : (ni + 1) * NCHUNK],
            in_=ps[:m_p, :],
        )
    if not use_scatter:
        records.append(None)
        # SWDGE (gpsimd) writes sustain far more bandwidth than HWDGE here,
        # and a 6-way ROW split (full-width 16 KiB descriptors, ~20 rows per
        # DMA) measures fastest: fewer HBM write receipts than column splits
        # while keeping several instructions in flight across SBUF ports.
        rb = [m_p * ri // nsp for ri in range(nsp + 1)]
        for ri in range(nsp):
            nc.gpsimd.dma_start(
                out=out_ext[
                    bi * f + o0 + rb[ri] : bi * f + o0 + rb[ri + 1], :
                ],
                in_=ot[rb[ri] : rb[ri + 1], :],
            )
        return
    records.append(sum(r is not None for r in records))
    # chunk 0 via plain SWDGE q0 write; chunks 1-3 via scatter on queues 1-3
    nc.gpsimd.dma_start(
        out=out_ext[bi * f + o0 : bi * f + o0 + m_p, 0:SCH],
        in_=ot[:m_p, 0:SCH],
    )
    out_v = out_ext[:].rearrange("r (a c) -> (r a) c", a=t // SCH)
    for ci in range(1, 4):
        icol = (col * 3 + (ci - 1)) * 8
        nc.gpsimd.dma_scatter_add(
            out_ap=out_v,
            in_ap=ot[:, ci * SCH : (ci + 1) * SCH].rearrange(
                "p (a e) -> p a e", a=1
            ),
            idxs_ap=sidx_tile[:, icol : icol + 8],
            num_idxs=124,
            num_idxs_reg=m_p,
            elem_size=SCH,
            queue_num=ci,
        )


def _build_banded_v2(b_shard, f, t, tile_m, n_tiles, reps=1, nsp=5,
                     xt_bufs=4, ot_bufs=9, ps_bufs=4, read_eng="gpsimd",
                     cast_group=2, cast_scalar=1, dual_cast=0):
    """Pre-permuted variant: the host lays x out in gathered-window
    order, so input reads are plain contiguous DMAs (no indirect
    gather machinery). Reads stay on SWDGE by default — mixing HWDGE
    reads with SWDGE writes measured 2x slower (ring-arbitration
    phasing starves the pipeline; engines sat ~70% idle)."""
    span = 129 - tile_m
    n_full = f // tile_m
    m_last = f - n_full * tile_m
    k_last = m_last + span - 1 if m_last else 0
    nc = bacc.Bacc()
    xwa_in = nc.declare_dram_parameter(
        "xwa", [128, b_shard * n_full * t], mybir.dt.bfloat16, isOutput=False
    )
    if m_last:
        xwb_in = nc.declare_dram_parameter(
            "xwb", [b_shard * k_last, t], mybir.dt.bfloat16, isOutput=False
        )
    wts_in = nc.declare_dram_parameter(
        "wts", [n_tiles, 128, tile_m], mybir.dt.bfloat16, isOutput=False
    )
    out_ext = nc.declare_dram_parameter(
        "out", [b_shard * f, t], mybir.dt.bfloat16, isOutput=True
    )
    n_chunks = t // NCHUNK
    with tile.TileContext(nc) as tc:
        with contextlib.ExitStack() as _st:
            cpool = _st.enter_context(tc.tile_pool(name="const", bufs=1))
            xpool = _st.enter_context(tc.tile_pool(name="xp", bufs=xt_bufs))
            if dual_cast:
                # fully separate per-engine pools: no cross-engine
                # resource coupling (shared pools convoy the in-order
                # engine queues)
                opools = [
                    _st.enter_context(
                        tc.tile_pool(name="opA", bufs=(ot_bufs + 1) // 2)
                    ),
                    _st.enter_context(
                        tc.tile_pool(name="opB", bufs=ot_bufs // 2)
                    ),
                ]
                pspools = [
                    _st.enter_context(
                        tc.tile_pool(name="psA", bufs=(ps_bufs + 1) // 2,
                                     space="PSUM")
                    ),
                    _st.enter_context(
                        tc.tile_pool(name="psB", bufs=ps_bufs // 2,
                                     space="PSUM")
                    ),
                ]
            else:
                opool = _st.enter_context(
                    tc.tile_pool(name="op", bufs=ot_bufs)
                )
                pspool = _st.enter_context(
                    tc.tile_pool(name="ps", bufs=ps_bufs, space="PSUM")
                )
                opools, pspools = [opool, opool], [pspool, pspool]
            wload_order = ([n_tiles - 1] if m_last else []) + list(range(n_full))
            w_tiles = [None] * n_tiles
            for p in wload_order:
                wt = cpool.tile([128, tile_m], mybir.dt.bfloat16, tag=f"w{p}")
                nc.sync.dma_start(out=wt[:], in_=wts_in[p])
                w_tiles[p] = wt
            k = 0
            rd = getattr(nc, read_eng)
            if dual_cast:
                _casts = [nc.vector.tensor_copy, nc.scalar.copy]
            elif cast_scalar:
                _casts = [nc.scalar.copy, nc.scalar.copy]
            else:
                _casts = [nc.vector.tensor_copy, nc.vector.tensor_copy]
            # partial window first: its 0.4 MB read lets compute
            # start ~6us earlier while the 2 MB pair reads stream in
            # underneath (shaves single-shot ramp; order-independent)
            p_order = ([n_tiles - 1] if m_last else []) + list(range(n_full))
            for _rep in range(reps):
                for bi in range(b_shard):
                    xt2 = None
                    for p in p_order:
                        o0 = p * tile_m
                        m_p = min(tile_m, f - o0)
                        k_need = m_p + span - 1
                        # paired reads: one 2-window DMA feeds two work
                        # items (halves read emissions on Q7)
                        if p < n_full:
                            if p % 2 == 0:
                                xt2 = xpool.tile(
                                    [128, 2 * t], mybir.dt.bfloat16, tag="xt"
                                )
                                off = (bi * n_full + p) * t
                                w = min(2, n_full - p)
                                rd.dma_start(
                                    out=xt2[:, : w * t],
                                    in_=xwa_in[:, off : off + w * t],
                                )
                            xsrc, xoff = xt2, (p % 2) * t
                        else:
                            xtp = xpool.tile(
                                [128, 2 * t], mybir.dt.bfloat16, tag="xt"
                            )
                            rd.dma_start(
                                out=xtp[:k_last, :t],
                                in_=xwb_in[bi * k_last : (bi + 1) * k_last, :],
                            )
                            xsrc, xoff = xtp, 0
                        side = k % 2 if dual_cast else 0
                        ot = opools[side].tile(
                            [128, t], mybir.dt.bfloat16, tag="ot"
                        )
                        # single cast engine: mixing DVE+ACT on this
                        # pipeline measured 2.2x slower (semaphore
                        # convoys between the in-order engine queues).
                        # cast_pair: two matmuls fill one 2-bank PSUM
                        # tile; one wide cast amortizes the DVE
                        # PSUM-read bubble (120 cyc) over 1024 elems.
                        cg = cast_group if n_chunks % cast_group == 0 else 1
                        for pi in range(n_chunks // cg):
                            ps = pspools[side].tile(
                                [128, cg * NCHUNK], mybir.dt.float32
                            )
                            for h in range(cg):
                                ni = cg * pi + h
                                nc.tensor.matmul(
                                    out=ps[:m_p, h * NCHUNK : (h + 1) * NCHUNK],
                                    lhsT=w_tiles[p][:k_need, :m_p],
                                    rhs=xsrc[
                                        :k_need,
                                        xoff + ni * NCHUNK : xoff + (ni + 1) * NCHUNK,
                                    ],
                                    start=True,
                                    stop=True,
                                )
                            _casts[side](
                                out=ot[:m_p, cg * pi * NCHUNK : (cg * pi + cg) * NCHUNK],
                                in_=ps[:m_p, :],
                            )
                        rb = [m_p * ri // nsp for ri in range(nsp + 1)]
                        for ri in range(nsp):
                            nc.gpsimd.dma_start(
                                out=out_ext[
                                    bi * f + o0 + rb[ri] : bi * f + o0 + rb[ri + 1], :
                                ],
                                in_=ot[rb[ri] : rb[ri + 1], :],
                            )
                        k += 1
    return nc


def _preperm_core(xb_core, perm, tile_m, n_tiles, f, t, d_hi):
    """Host: build (xwa, xwb) window-layout inputs for one core.
    xb_core: [b_shard, f, t] bf16."""
    b_shard = xb_core.shape[0]
    span = 129 - tile_m
    n_full = f // tile_m
    karr = np.arange(128)
    rfull = np.stack(
        [perm[(p * tile_m - d_hi + karr) % f] for p in range(n_full)]
    )  # [n_full, 128]
    m_last = f - n_full * tile_m
    k_last = m_last + span - 1 if m_last else 0
    xwa_parts, xwb_parts = [], []
    for b in range(b_shard):
        g = xb_core[b][rfull.reshape(-1)]  # [n_full*128, t]
        xwa_parts.append(g.reshape(n_full, 128, t).transpose(1, 0, 2))
        if m_last:
            rows_last = (n_full * tile_m - d_hi + np.arange(k_last)) % f
            xwb_parts.append(xb_core[b][perm[rows_last]])
    xwa = np.ascontiguousarray(
        np.concatenate(xwa_parts, axis=1).reshape(128, b_shard * n_full * t)
    )
    xwb = (
        np.ascontiguousarray(np.concatenate(xwb_parts, axis=0))
        if m_last
        else None
    )
    return xwa, xwb


def _build_dense_program(b_shard, f, t):
    """Fallback: generic block matmul out_p = sum_q M[p,q] @ xp_q.

    Splits T in halves to fit 8 resident gathered input tiles in SBUF.
    """
    nc = bacc.Bacc()
    nq = f // 128
    x_in = nc.declare_dram_parameter("x", [b_shard * f, t], mybir.dt.float32, isOutput=False)
    wts_in = nc.declare_dram_parameter(
        "wts", [nq, nq, 128, 128], mybir.dt.float32, isOutput=False
    )
    gidx_in = nc.declare_dram_parameter(
        "gidx", [128, b_shard * nq], mybir.dt.int32, isOutput=False
    )
    out_ext = nc.declare_dram_parameter(
        "out", [b_shard * f, t], mybir.dt.float32, isOutput=True
    )
    t_half = t // 2
    n_chunks = t_half // NCHUNK
    with tile.TileContext(nc) as tc:
        with (
            tc.tile_pool(name="const", bufs=1) as cpool,
            tc.tile_pool(name="xp", bufs=10) as xpool,
            tc.tile_pool(name="op", bufs=2) as opool,
            tc.tile_pool(name="ps", bufs=6, space="PSUM") as pspool,
        ):
            idx_tile = cpool.tile([128, b_shard * nq], mybir.dt.int32)
            nc.sync.dma_start(out=idx_tile[:], in_=gidx_in[:])
            w_tiles = {}
            for p in range(nq):
                for q in range(nq):
                    wt = cpool.tile([128, 128], mybir.dt.float32, tag=f"w{p}_{q}")
                    nc.sync.dma_start(out=wt[:], in_=wts_in[p, q])
                    w_tiles[(p, q)] = wt
            for bi in range(b_shard):
                for th in range(2):
                    t0 = th * t_half
                    xts = []
                    for q in range(nq):
                        col = bi * nq + q
                        xt = xpool.tile([128, t_half], mybir.dt.float32, tag="xt")
                        nc.gpsimd.indirect_dma_start(
                            out=xt[:],
                            out_offset=None,
                            in_=x_in[:],
                            in_offset=bass.IndirectOffsetOnAxis(
                                ap=idx_tile[:, col : col + 1], axis=0
                            ),
                            element_offset=t0,
                        )
                        xts.append(xt)
                    for p in range(nq):
                        ot = opool.tile([128, t_half], mybir.dt.float32, tag="ot")
                        for ni in range(n_chunks):
                            ps = pspool.tile([128, NCHUNK], mybir.dt.float32)
                            for q in range(nq):
                                nc.tensor.matmul(
                                    out=ps[:, :],
                                    lhsT=w_tiles[(p, q)][:],
                                    rhs=xts[q][:, ni * NCHUNK : (ni + 1) * NCHUNK],
                                    start=(q == 0),
                                    stop=(q == nq - 1),
                                )
                            nc.vector.tensor_copy(
                                out=ot[:, ni * NCHUNK : (ni + 1) * NCHUNK],
                                in_=ps[:, :],
                            )
                        nc.sync.dma_start(
                            out=out_ext[
                                bi * f + p * 128 : bi * f + (p + 1) * 128,
                                t0 : t0 + t_half,
                            ],
                            in_=ot[:, :],
                        )
    return nc


def _build_sidx(b_shard, f, t, tile_m, n_tiles):
    """int16 scatter indices for dma_scatter_add output writes: view out as
    [(rows*t/SCH), SCH]; tile (bi,p) chunk ci covers view-rows
    (bi*f+o0+r)*(t//SCH)+ci, 16-partition-wrapped, -1 trailing padding."""
    a = t // SCH
    sidx = np.full((128, b_shard * n_tiles * 3 * 8), -1, np.int16)
    pp = np.arange(128)
    for bi in range(b_shard):
        for p in range(n_tiles):
            o0 = p * tile_m
            m_p = min(tile_m, f - o0)
            col = bi * n_tiles + p
            for ci in range(1, 4):
                vals = np.full(128, -1, np.int64)
                vals[:m_p] = (bi * f + o0 + np.arange(m_p)) * a + ci
                icol = (col * 3 + (ci - 1)) * 8
                for s in range(8):
                    sidx[:, icol + s] = vals[s * 16 + pp % 16]
    return sidx


LAST_DHI = 0  # band upper shift from the most recent _analyze


def _analyze(mixer, permutation, b_shard, f):
    """Derive band structure + weights/indices. Returns (mode, tile_m,
    n_tiles, wts, gidx)."""
    global LAST_DHI
    perm = permutation.astype(np.int64)
    o_idx, c_idx = np.nonzero(mixer)
    if len(o_idx) == 0:
        d_lo = d_hi = 0
    else:
        d = (o_idx - c_idx) % f
        d_signed = np.where(d > f // 2, d - f, d)
        d_lo, d_hi = int(d_signed.min()), int(d_signed.max())
    span = d_hi - d_lo + 1
    if span <= 128:
        tile_m = 129 - span
        n_tiles = -(-f // tile_m)
        wts = np.zeros((n_tiles, 128, tile_m), np.float32)
        gidx = np.zeros((128, b_shard * n_tiles), np.int32)
        a_hat = np.zeros((f, f), np.float32)
        k_arange = np.arange(128)
        for p in range(n_tiles):
            o0 = p * tile_m
            m_p = min(tile_m, f - o0)
            rows = (o0 - d_hi + k_arange) % f  # feature index i per window row
            wts[p, :, :m_p] = mixer[np.ix_(range(o0, o0 + m_p), rows)].T
            a_hat[np.ix_(range(o0, o0 + m_p), rows)] = wts[p, :, :m_p].T
            k_need = m_p + (129 - tile_m) - 1  # window rows actually used
            for bi in range(b_shard):
                col = bi * n_tiles + p
                gidx[:, col] = bi * f + perm[rows]
                # out-of-bounds sentinel: gather skips these rows entirely
                gidx[k_need:, col] = b_shard * f
        if np.array_equal(a_hat, mixer):
            LAST_DHI = d_hi
            return ("banded", tile_m, n_tiles, wts, gidx)
    # dense fallback
    nq = f // 128
    wts = np.ascontiguousarray(
        mixer.reshape(nq, 128, nq, 128).transpose(0, 2, 3, 1), dtype=np.float32
    )
    gidx = np.zeros((128, b_shard * nq), np.int32)
    for bi in range(b_shard):
        for q in range(nq):
            gidx[:, bi * nq + q] = bi * f + perm[q * 128 : (q + 1) * 128]
    return ("dense", 128, nq, wts, gidx)


def kernel(x, mixer, permutation):
    global LAST_RESULTS
    x = np.ascontiguousarray(x, dtype=np.float32)
    mixer = np.asarray(mixer, dtype=np.float32)
    permutation = np.asarray(permutation)
    b, f, t = x.shape
    b_shard = b // N_CORES

    mode, tile_m, n_tiles, wts, gidx = _analyze(mixer, permutation, b_shard, f)

    variant = "v2" if USE_V2 else "v1"
    key = (mode, b_shard, f, t, tile_m, n_tiles, variant)
    if key not in _PROGRAM_CACHE:
        if mode == "banded" and USE_V2:
            _PROGRAM_CACHE[key] = _build_banded_v2(
                b_shard, f, t, tile_m, n_tiles
            )
        elif mode == "banded":
            _PROGRAM_CACHE[key] = _build_banded_program(
                b_shard, f, t, tile_m, n_tiles, scatter_out=False
            )
        else:
            _PROGRAM_CACHE[key] = _build_dense_program(b_shard, f, t)
    nc = _PROGRAM_CACHE[key]
    if not getattr(nc, "_lowbit_compiled", False):
        nc.compile()
        nc._lowbit_compiled = True

    # banded path runs bf16 end-to-end: mixer weights are low-bit
    # (powers of two, exact in bf16) so the only loss is input/output
    # quantization (~2.5e-3 rel), and HBM traffic halves. In v2 the
    # host pre-permutes x into gathered-window order so device reads
    # are plain contiguous DMAs.
    in_maps = []
    if mode == "banded" and USE_V2:
        perm64 = permutation.astype(np.int64)
        wts_ship = wts.astype(ml_dtypes.bfloat16)
        for i in range(N_CORES):
            xb_core = x[i * b_shard : (i + 1) * b_shard].astype(
                ml_dtypes.bfloat16
            )
            xwa, xwb = _preperm_core(
                xb_core, perm64, tile_m, n_tiles, f, t, LAST_DHI
            )
            m = {"xwa": xwa, "wts": wts_ship}
            if xwb is not None:
                m["xwb"] = xwb
            in_maps.append(m)
    else:
        ship_dt = ml_dtypes.bfloat16 if mode == "banded" else np.float32
        wts_ship = wts.astype(ship_dt)
        for i in range(N_CORES):
            in_maps.append({
                "x": np.ascontiguousarray(
                    x[i * b_shard : (i + 1) * b_shard].reshape(b_shard * f, t)
                ).astype(ship_dt),
                "wts": wts_ship,
                "gidx": gidx,
            })
    global LAST_NC, LAST_IN_MAPS
    LAST_NC = nc
    LAST_IN_MAPS = in_maps
    res = run_bass_kernel_spmd(nc, in_maps, list(range(N_CORES)))
    LAST_RESULTS = res
    out = np.concatenate(
        [r["out"].astype(np.float32).reshape(b_shard, f, t) for r in res.results],
        axis=0,
    )
    return out

